# revision 5
# baseline (speedup 1.0000x reference)
"""Multi-head attention (B=2, S=2048, H=1024, 16 heads x 64) on 8 trn2 cores.

Sharding: data-parallel over batch (2) x tensor-parallel over heads (4 groups
of 4 heads). Core c handles batch c//4, head-group c%4 (wq/wk/wv columns
[256*g, 256*g+256)). Host slices inputs per core and concatenates the
per-core head-slice outputs.

Per-core pipeline (all fp32 data, fp32r matmuls):
  1. PE-transpose q,k,v [2048,1024] -> xT [1024,2048] (128x128 blocks).
  2. Projections (fp32r): QT/KT = (x @ w + b)^T as [256, 2048] tiles with
     head-pairs stacked on partitions (64+64); V-chunks transposed back to
     VH' [128,129] per S-tile: [headA(64) | ones | headB(64)] - the shared
     ones column yields softmax denominators in the PV matmul for free.
  3. Attention per (q-tile of 512, head-pair): scores transposed
     ST[keys,q] via K=64 row-packed matmul pairs (tile_position (0,0) and
     (64,0)); exp on ACT (scale=1/32, no max subtraction - scores are
     O(1) by construction); PV accumulates out'^T [65, 512] in PSUM over
     16 key tiles. Then PE-transpose to [q,65], divide by the denominator
     column, stage 4 heads side by side, DMA out.

The softmax mask of the reference is a mathematical no-op (it broadcasts
over the key axis, shifting every logit of a row equally), so it is ignored.
"""

import numpy as np

B, S, H = 2, 2048, 1024
NH, D = 16, 64            # heads, head_dim
CORES = 8
GROUP_COLS = 256          # 4 heads per core
SCALE = 1.0 / 32.0        # 1/sqrt(H)

_CACHE = {}


def _build():
    import concourse.bacc as bacc
    import concourse.tile as tile
    import concourse.mybir as mybir
    from concourse.masks import make_identity

    F32 = mybir.dt.float32
    F32R = mybir.dt.float32r
    EXP = mybir.ActivationFunctionType.Exp

    nc = bacc.Bacc("TRN2", target_bir_lowering=False, debug=False,
                   num_devices=CORES)

    q_d = nc.dram_tensor("q", [S, H], F32, kind="ExternalInput").ap()
    k_d = nc.dram_tensor("k", [S, H], F32, kind="ExternalInput").ap()
    v_d = nc.dram_tensor("v", [S, H], F32, kind="ExternalInput").ap()
    w_d = {x: nc.dram_tensor("w" + x, [H, GROUP_COLS], F32,
                             kind="ExternalInput").ap() for x in "qkv"}
    b_d = {x: nc.dram_tensor("b" + x, [GROUP_COLS, 1], F32,
                             kind="ExternalInput").ap() for x in "qkv"}
    out_d = nc.dram_tensor("out", [S, GROUP_COLS], F32,
                           kind="ExternalOutput").ap()
    x_d = {"q": q_d, "k": k_d, "v": v_d}

    NS = S // 128          # 16 S-tiles
    NK = H // 128          # 8 K-tiles (contraction over H)
    NQ = S // 512          # 4 q-tiles of 512
    NM = 2                 # head-pairs per core

    from contextlib import ExitStack

    with tile.TileContext(nc) as tc, ExitStack() as es:
        const = es.enter_context(tc.tile_pool(name="const", bufs=1))
        wpool = es.enter_context(tc.tile_pool(name="w", bufs=1))
        nat = es.enter_context(tc.tile_pool(name="nat", bufs=3))
        xT = es.enter_context(tc.tile_pool(name="xT", bufs=1))
        proj = es.enter_context(tc.tile_pool(name="proj", bufs=1))
        vchunkp = es.enter_context(tc.tile_pool(name="vchunk", bufs=2))
        vhp = es.enter_context(tc.tile_pool(name="vh", bufs=1))
        pexpp = es.enter_context(tc.tile_pool(name="pexp", bufs=2))
        pvsbp = es.enter_context(tc.tile_pool(name="pvsb", bufs=4))
        stagep = es.enter_context(tc.tile_pool(name="stage", bufs=8))
        recp = es.enter_context(tc.tile_pool(name="rec", bufs=8))
        ps_st = es.enter_context(tc.tile_pool(name="ps_st", bufs=2, space="PSUM"))
        ps_tr = es.enter_context(tc.tile_pool(name="ps_tr", bufs=2, space="PSUM"))
        ps_pv = es.enter_context(tc.tile_pool(name="ps_pv", bufs=1, space="PSUM"))

        ident = const.tile([128, 128], F32, tag="ident")
        make_identity(nc, ident[:])
        ones = const.tile([128, 1], F32, tag="ones")
        nc.vector.memset(ones[:], 1.0)

        bias_t = {}
        for x in "qkv":
            for m in range(NM):
                bt = const.tile([128, 1], F32, tag=f"b{x}{m}")
                nc.sync.dma_start(out=bt[:], in_=b_d[x][128 * m:128 * m + 128, :])
                bias_t[(x, m)] = bt

        # persistent projection outputs
        QT = [proj.tile([128, S], F32R, tag=f"qt{m}", name=f"QT{m}") for m in range(NM)]
        KT = [proj.tile([128, S], F32R, tag=f"kt{m}", name=f"KT{m}") for m in range(NM)]
        VH = [[vhp.tile([128, 129], F32R, tag=f"vh{m}_{s}", name=f"VH{m}_{s}") for s in range(NS)]
              for m in range(NM)]

        def proj_phase(x):
            """Load + transpose input x, project to QT/KT or VH'."""
            rw = []
            for kb in range(NK):
                wt = wpool.tile([128, GROUP_COLS], F32, tag=f"w{kb}")
                nc.sync.dma_start(out=wt[:], in_=w_d[x][128 * kb:128 * kb + 128, :])
                rwt = wpool.tile([128, GROUP_COLS], F32R, tag=f"rw{kb}")
                nc.vector.tensor_copy(rwt[:], wt[:])
                rw.append(rwt)

            xTt = [xT.tile([128, S], F32R, tag=f"t{kb}", name=f"xTt{kb}") for kb in range(NK)]
            for s in range(NS):
                natt = nat.tile([128, H], F32, tag="nat")
                nc.sync.dma_start(out=natt[:], in_=x_d[x][128 * s:128 * s + 128, :])
                for kb in range(NK):
                    trp = ps_tr.tile([128, 128], F32, tag="tr")
                    nc.tensor.transpose(trp[:], natt[:, 128 * kb:128 * kb + 128],
                                        ident[:])
                    nc.vector.tensor_copy(xTt[kb][:, 128 * s:128 * s + 128], trp[:])

            for m in range(NM):
                for nt in range(NQ):
                    acc = ps_st.tile([128, 1024], F32, tag="st")
                    a = acc[:, 0:512]
                    for kb in range(NK):
                        nc.tensor.matmul(
                            a, rw[kb][:, 128 * m:128 * m + 128],
                            xTt[kb][:, 512 * nt:512 * nt + 512],
                            start=(kb == 0), stop=(kb == NK - 1))
                    if x != "v":
                        dst = (QT if x == "q" else KT)[m][:, 512 * nt:512 * nt + 512]
                        nc.scalar.add(dst, a, bias_t[(x, m)][:, 0:1])
                    else:
                        vchunk = vchunkp.tile([128, 512], F32, tag="vchunk")
                        nc.scalar.add(vchunk[:], a, bias_t[(x, m)][:, 0:1])
                        for i in range(4):
                            s = 4 * nt + i
                            trp = ps_tr.tile([128, 128], F32, tag="tr")
                            nc.tensor.transpose(
                                trp[:], vchunk[:, 128 * i:128 * i + 128], ident[:])
                            vt = VH[m][s]
                            nc.vector.tensor_copy(vt[:, 0:64], trp[:, 0:64])
                            nc.vector.tensor_copy(vt[:, 65:129], trp[:, 64:128])
                            nc.vector.tensor_copy(vt[:, 64:65], ones[:])

        proj_phase("q")
        proj_phase("k")
        proj_phase("v")

        # ---- attention ----
        for qt in range(NQ):
            stage = [stagep.tile([128, GROUP_COLS], F32, tag="stage",
                                  name=f"stage{qt}_{i}")
                     for i in range(4)]
            for m in range(NM):
                pva = ps_pv.tile([65, 512], F32, tag="pva")
                pvb = ps_pv.tile([65, 512], F32, tag="pvb")

                def emit_scores(kt):
                    stt = ps_st.tile([128, 1024], F32, tag="st")
                    nc.tensor.matmul(
                        stt[:, 0:512],
                        KT[m][0:64, 128 * kt:128 * kt + 128],
                        QT[m][0:64, 512 * qt:512 * qt + 512],
                        start=True, stop=True)
                    nc.tensor.matmul(
                        stt[:, 512:1024],
                        KT[m][64:128, 128 * kt:128 * kt + 128],
                        QT[m][64:128, 512 * qt:512 * qt + 512],
                        start=True, stop=True, tile_position=(64, 0))
                    pe = pexpp.tile([128, 1024], F32R, tag="pexp")
                    nc.scalar.activation(pe[:], stt[:], EXP, scale=SCALE)
                    return pe

                pending = emit_scores(0)
                for kt in range(NS):
                    nxt = emit_scores(kt + 1) if kt + 1 < NS else None
                    pe = pending
                    nc.tensor.matmul(pva[:], VH[m][kt][:, 0:65], pe[:, 0:512],
                                     start=(kt == 0), stop=(kt == NS - 1))
                    nc.tensor.matmul(pvb[:], VH[m][kt][:, 64:129], pe[:, 512:1024],
                                     start=(kt == 0), stop=(kt == NS - 1))
                    pending = nxt

                sba = pvsbp.tile([65, 512], F32, tag="pvsb")
                nc.vector.tensor_copy(sba[:], pva[:])
                sbb = pvsbp.tile([65, 512], F32, tag="pvsb")
                nc.vector.tensor_copy(sbb[:], pvb[:])
                for sub in range(4):
                    tra = ps_tr.tile([128, 128], F32, tag="tr")
                    nc.tensor.transpose(tra[:, 0:65],
                                        sba[0:65, 128 * sub:128 * sub + 128],
                                        ident[0:65, 0:65])
                    ra = recp.tile([128, 1], F32, tag="rec")
                    nc.vector.reciprocal(ra[:], tra[:, 64:65])
                    nc.vector.tensor_scalar_mul(
                        stage[sub][:, 128 * m:128 * m + 64],
                        tra[:, 0:64], ra[:, 0:1])

                    trb = ps_tr.tile([128, 128], F32, tag="tr")
                    nc.tensor.transpose(trb[:, 0:65],
                                        sbb[0:65, 128 * sub:128 * sub + 128],
                                        ident[0:65, 0:65])
                    rb = recp.tile([128, 1], F32, tag="rec")
                    nc.vector.reciprocal(rb[:], trb[:, 0:1])
                    nc.vector.tensor_scalar_mul(
                        stage[sub][:, 128 * m + 64:128 * m + 128],
                        trb[:, 1:65], rb[:, 0:1])

            for sub in range(4):
                nc.sync.dma_start(
                    out=out_d[512 * qt + 128 * sub:512 * qt + 128 * sub + 128, :],
                    in_=stage[sub][:])

    nc.compile()
    return nc


def _get_nc():
    if "nc" not in _CACHE:
        _CACHE["nc"] = _build()
    return _CACHE["nc"]


def _run(inputs, trace=False, tmpdir=None):
    from concourse.bass_utils import run_bass_kernel_spmd

    nc = _get_nc()
    q, k, v = inputs["q"], inputs["k"], inputs["v"]
    wq, wk, wv = inputs["wq"], inputs["wk"], inputs["wv"]
    bq, bk, bv = inputs["bq"], inputs["bk"], inputs["bv"]

    def f32(a):
        return np.ascontiguousarray(np.asarray(a), dtype=np.float32)

    in_maps = []
    for c in range(CORES):
        b, g = divmod(c, CORES // B)
        sel = slice(GROUP_COLS * g, GROUP_COLS * g + GROUP_COLS)
        in_maps.append({
            "q": f32(q[b]), "k": f32(k[b]), "v": f32(v[b]),
            "wq": f32(wq[:, sel]), "wk": f32(wk[:, sel]), "wv": f32(wv[:, sel]),
            "bq": f32(bq[sel]).reshape(GROUP_COLS, 1),
            "bk": f32(bk[sel]).reshape(GROUP_COLS, 1),
            "bv": f32(bv[sel]).reshape(GROUP_COLS, 1),
        })

    res = run_bass_kernel_spmd(nc, in_maps, list(range(CORES)),
                               trace=trace, tmpdir=tmpdir)
    out = np.empty((B, S, H), dtype=np.float32)
    for c in range(CORES):
        b, g = divmod(c, CORES // B)
        out[b, :, GROUP_COLS * g:GROUP_COLS * g + GROUP_COLS] = \
            res.results[c]["out"]
    return out, res


def kernel(**inputs):
    out, _ = _run(inputs, trace=False)
    return out


# revision 6
# speedup vs baseline: 1.3500x; 1.3500x over previous
"""Multi-head attention (B=2, S=2048, H=1024, 16 heads x 64) on 8 trn2 cores.

Sharding: data-parallel over batch (2) x tensor-parallel over heads (4 groups
of 4 heads). Core c handles batch c//4, head-group c%4 (wq/wk/wv columns
[256*g, 256*g+256)). Host slices inputs per core (shipping q/k/v pre-cast to
bf16 - the kernel's chosen compute precision) and concatenates the per-core
head-slice outputs.

Per-core pipeline (bf16 matmul operands, fp32 PSUM accumulation):
  1. q,k,v arrive [S,H] bf16 in DRAM; DMA-xbar-transpose loads them directly
     as xT [H-partition, S] SBUF tiles (no PE transposes, no casts on device).
  2. Projections: QT/KT = (x @ w + b)^T as [256, 2048] bf16 tiles with
     head-pairs stacked on partitions (64+64); V-chunks transposed back to
     VH' [128,129] per S-tile: [headA(64) | ones | headB(64)] - the shared
     ones column yields softmax denominators in the PV matmul for free.
  3. Attention per (q-tile of 512, head-pair): scores transposed ST[keys,q]
     via K=64 row-packed matmul pairs (tile_position (0,0) and (64,0));
     exp on ACT (scale=1/32, no max subtraction - logits are O(0.25) by
     construction); PV accumulates out'^T [65, 512] in PSUM over 16 key
     tiles. Then PE-transpose to [q,65] (f32), divide by the denominator
     column, stage 4 heads side by side, DMA out (f32).

The softmax mask of the reference is a mathematical no-op (it broadcasts
over the key axis, shifting every logit of a row equally), so it is ignored.
"""

import numpy as np

B, S, H = 2, 2048, 1024
NH, D = 16, 64            # heads, head_dim
CORES = 8
GROUP_COLS = 256          # 4 heads per core
SCALE = 1.0 / 32.0        # 1/sqrt(H)

_CACHE = {}


def _build():
    import concourse.bacc as bacc
    import concourse.tile as tile
    import concourse.mybir as mybir
    from concourse.masks import make_identity
    from contextlib import ExitStack

    F32 = mybir.dt.float32
    BF16 = mybir.dt.bfloat16
    EXP = mybir.ActivationFunctionType.Exp

    nc = bacc.Bacc("TRN2", target_bir_lowering=False, debug=False,
                   num_devices=CORES)

    q_d = nc.dram_tensor("q", [S, H], BF16, kind="ExternalInput").ap()
    k_d = nc.dram_tensor("k", [S, H], BF16, kind="ExternalInput").ap()
    v_d = nc.dram_tensor("v", [S, H], BF16, kind="ExternalInput").ap()
    w_d = {x: nc.dram_tensor("w" + x, [H, GROUP_COLS], F32,
                             kind="ExternalInput").ap() for x in "qkv"}
    b_d = {x: nc.dram_tensor("b" + x, [GROUP_COLS, 1], F32,
                             kind="ExternalInput").ap() for x in "qkv"}
    out_d = nc.dram_tensor("out", [S, GROUP_COLS], F32,
                           kind="ExternalOutput").ap()
    x_d = {"q": q_d, "k": k_d, "v": v_d}

    NS = S // 128          # 16 S-tiles
    NK = H // 128          # 8 K-tiles (contraction over H)
    NQ = S // 512          # 4 q-tiles of 512
    NM = 2                 # head-pairs per core

    with tile.TileContext(nc) as tc, ExitStack() as es:
        const = es.enter_context(tc.tile_pool(name="const", bufs=1))
        wpool = es.enter_context(tc.tile_pool(name="w", bufs=1))
        xT = es.enter_context(tc.tile_pool(name="xT", bufs=1))
        proj = es.enter_context(tc.tile_pool(name="proj", bufs=1))
        vchunkp = es.enter_context(tc.tile_pool(name="vchunk", bufs=2))
        vhp = es.enter_context(tc.tile_pool(name="vh", bufs=1))
        pexpp = es.enter_context(tc.tile_pool(name="pexp", bufs=3))
        pvsbp = es.enter_context(tc.tile_pool(name="pvsb", bufs=4))
        stagep = es.enter_context(tc.tile_pool(name="stage", bufs=8))
        recp = es.enter_context(tc.tile_pool(name="rec", bufs=8))
        ps_st = es.enter_context(tc.tile_pool(name="ps_st", bufs=2, space="PSUM"))
        ps_tr = es.enter_context(tc.tile_pool(name="ps_tr", bufs=2, space="PSUM"))
        ps_pv = es.enter_context(tc.tile_pool(name="ps_pv", bufs=1, space="PSUM"))

        ident = const.tile([128, 128], F32, tag="ident")
        make_identity(nc, ident[:])
        identb = const.tile([128, 128], BF16, tag="identb")
        make_identity(nc, identb[:])

        bias_t = {}
        for x in "qkv":
            for m in range(NM):
                bt = const.tile([128, 1], F32, tag=f"b{x}{m}")
                nc.sync.dma_start(out=bt[:], in_=b_d[x][128 * m:128 * m + 128, :])
                bias_t[(x, m)] = bt

        # persistent projection outputs
        QT = [proj.tile([128, S], BF16, tag=f"qt{m}", name=f"QT{m}")
              for m in range(NM)]
        KT = [proj.tile([128, S], BF16, tag=f"kt{m}", name=f"KT{m}")
              for m in range(NM)]
        VH = [[vhp.tile([128, 129], BF16, tag=f"vh{m}_{s}", name=f"VH{m}_{s}")
               for s in range(NS)] for m in range(NM)]

        def proj_phase(x):
            """xbar-transpose-load input x, project to QT/KT or VH'."""
            wbf = []
            for kb in range(NK):
                wt = wpool.tile([128, GROUP_COLS], F32, tag=f"w{kb}")
                nc.sync.dma_start(out=wt[:], in_=w_d[x][128 * kb:128 * kb + 128, :])
                wb = wpool.tile([128, GROUP_COLS], BF16, tag=f"wb{kb}")
                nc.vector.tensor_copy(wb[:], wt[:])
                wbf.append(wb)

            xTt = [xT.tile([128, S], BF16, tag=f"t{kb}", name=f"xTt{kb}")
                   for kb in range(NK)]
            for kb in range(NK):
                nc.sync.dma_start_transpose(
                    out=xTt[kb][:], in_=x_d[x][:, 128 * kb:128 * kb + 128])

            for m in range(NM):
                for nt in range(NQ):
                    acc = ps_st.tile([128, 1024], F32, tag="st")
                    a = acc[:, 0:512]
                    for kb in range(NK):
                        nc.tensor.matmul(
                            a, wbf[kb][:, 128 * m:128 * m + 128],
                            xTt[kb][:, 512 * nt:512 * nt + 512],
                            start=(kb == 0), stop=(kb == NK - 1))
                    if x != "v":
                        dst = (QT if x == "q" else KT)[m][:, 512 * nt:512 * nt + 512]
                        nc.vector.tensor_scalar_add(dst, a, bias_t[(x, m)][:, 0:1])
                    else:
                        vchunk = vchunkp.tile([128, 512], BF16, tag="vchunk")
                        nc.vector.tensor_scalar_add(vchunk[:], a,
                                                    bias_t[(x, m)][:, 0:1])
                        for i in range(4):
                            s = 4 * nt + i
                            trp = ps_tr.tile([128, 128], BF16, tag="tr",
                                             name="trv")
                            nc.tensor.transpose(
                                trp[:], vchunk[:, 128 * i:128 * i + 128],
                                identb[:])
                            vt = VH[m][s]
                            nc.vector.tensor_copy(vt[:, 0:64], trp[:, 0:64])
                            nc.vector.tensor_copy(vt[:, 65:129], trp[:, 64:128])
                            nc.vector.memset(vt[:, 64:65], 1.0)

        proj_phase("q")
        proj_phase("k")
        proj_phase("v")

        # ---- attention ----
        for qt in range(NQ):
            stage = [stagep.tile([128, GROUP_COLS], F32, tag="stage",
                                 name=f"stage{qt}_{i}") for i in range(4)]
            for m in range(NM):
                pva = ps_pv.tile([65, 512], F32, tag="pva")
                pvb = ps_pv.tile([65, 512], F32, tag="pvb")

                def emit_scores(kt):
                    stt = ps_st.tile([128, 1024], F32, tag="st")
                    nc.tensor.matmul(
                        stt[:, 0:512],
                        KT[m][0:64, 128 * kt:128 * kt + 128],
                        QT[m][0:64, 512 * qt:512 * qt + 512],
                        start=True, stop=True)
                    nc.tensor.matmul(
                        stt[:, 512:1024],
                        KT[m][64:128, 128 * kt:128 * kt + 128],
                        QT[m][64:128, 512 * qt:512 * qt + 512],
                        start=True, stop=True, tile_position=(64, 0))
                    pe = pexpp.tile([128, 1024], BF16, tag="pexp")
                    nc.scalar.activation(pe[:], stt[:], EXP, scale=SCALE)
                    return pe

                pending = emit_scores(0)
                for kt in range(NS):
                    nxt = emit_scores(kt + 1) if kt + 1 < NS else None
                    pe = pending
                    nc.tensor.matmul(pva[:], VH[m][kt][:, 0:65], pe[:, 0:512],
                                     start=(kt == 0), stop=(kt == NS - 1))
                    nc.tensor.matmul(pvb[:], VH[m][kt][:, 64:129],
                                     pe[:, 512:1024],
                                     start=(kt == 0), stop=(kt == NS - 1))
                    pending = nxt

                sba = pvsbp.tile([65, 512], F32, tag="pvsb")
                nc.vector.tensor_copy(sba[:], pva[:])
                sbb = pvsbp.tile([65, 512], F32, tag="pvsb")
                nc.vector.tensor_copy(sbb[:], pvb[:])
                for sub in range(4):
                    tra = ps_tr.tile([128, 128], F32, tag="tr", name="tra")
                    nc.tensor.transpose(tra[:, 0:65],
                                        sba[0:65, 128 * sub:128 * sub + 128],
                                        ident[0:65, 0:65])
                    ra = recp.tile([128, 1], F32, tag="rec", name="ra")
                    nc.vector.reciprocal(ra[:], tra[:, 64:65])
                    nc.vector.tensor_scalar_mul(
                        stage[sub][:, 128 * m:128 * m + 64],
                        tra[:, 0:64], ra[:, 0:1])

                    trb = ps_tr.tile([128, 128], F32, tag="tr", name="trb")
                    nc.tensor.transpose(trb[:, 0:65],
                                        sbb[0:65, 128 * sub:128 * sub + 128],
                                        ident[0:65, 0:65])
                    rb = recp.tile([128, 1], F32, tag="rec", name="rb")
                    nc.vector.reciprocal(rb[:], trb[:, 0:1])
                    nc.vector.tensor_scalar_mul(
                        stage[sub][:, 128 * m + 64:128 * m + 128],
                        trb[:, 1:65], rb[:, 0:1])

            for sub in range(4):
                nc.sync.dma_start(
                    out=out_d[512 * qt + 128 * sub:512 * qt + 128 * sub + 128, :],
                    in_=stage[sub][:])

    nc.compile()
    return nc


def _get_nc():
    if "nc" not in _CACHE:
        _CACHE["nc"] = _build()
    return _CACHE["nc"]


def _run(inputs, trace=False, tmpdir=None):
    import ml_dtypes
    from concourse.bass_utils import run_bass_kernel_spmd

    nc = _get_nc()
    q, k, v = inputs["q"], inputs["k"], inputs["v"]
    wq, wk, wv = inputs["wq"], inputs["wk"], inputs["wv"]
    bq, bk, bv = inputs["bq"], inputs["bk"], inputs["bv"]

    def f32(a):
        return np.ascontiguousarray(np.asarray(a), dtype=np.float32)

    def bf16(a):
        return np.ascontiguousarray(
            np.asarray(a, dtype=np.float32).astype(ml_dtypes.bfloat16))

    in_maps = []
    for c in range(CORES):
        b, g = divmod(c, CORES // B)
        sel = slice(GROUP_COLS * g, GROUP_COLS * g + GROUP_COLS)
        in_maps.append({
            "q": bf16(q[b]), "k": bf16(k[b]), "v": bf16(v[b]),
            "wq": f32(wq[:, sel]), "wk": f32(wk[:, sel]), "wv": f32(wv[:, sel]),
            "bq": f32(bq[sel]).reshape(GROUP_COLS, 1),
            "bk": f32(bk[sel]).reshape(GROUP_COLS, 1),
            "bv": f32(bv[sel]).reshape(GROUP_COLS, 1),
        })

    res = run_bass_kernel_spmd(nc, in_maps, list(range(CORES)),
                               trace=trace, tmpdir=tmpdir)
    out = np.empty((B, S, H), dtype=np.float32)
    for c in range(CORES):
        b, g = divmod(c, CORES // B)
        out[b, :, GROUP_COLS * g:GROUP_COLS * g + GROUP_COLS] = \
            res.results[c]["out"]
    return out, res


def kernel(**inputs):
    out, _ = _run(inputs, trace=False)
    return out
